# revision 4
# baseline (speedup 1.0000x reference)
"""Trainium2 Bass kernel for nn_DetectUDPModel (rank-2 Hermitian detection loss).

Math: the reference computes
    loss = sum_m |v_m|^2,   v_m = lam0 * u0^T B_m conj(u0) - lam1 * u1^T B_m conj(u1)
with B_m = R_m - i*I_m (basis_re/basis_im) and u_j the prepared eigenvectors.
Writing u = ur + i*ui and defining the two real row-vectors over b
    g1 = ur^T R + ui^T I
    g2 = ui^T R - ur^T I
one checks  u^T (R - iI) conj(u) = (g1 + i*g2) . conj(u)  exactly.  So per m
and per eigenvector only TWO device-side output rows are needed, and the R/I
pair is contracted JOINTLY -- a doubled contraction dim (256) that maps onto
the TensorEngine's fp8 DoubleRow perf mode (0.5 cycles/row, 2 k-tiles).

Device work (memory-bound streaming of the basis, fp8):
  The basis is cast to fp8-e4m3 on the host (quarters the DMA bytes vs f32)
  and packed so each chunk DMA is a fully contiguous HBM read landing
  partition-outer in SBUF with the R-plane and I-plane of each 4-m slot side
  by side.  Stationary planes hold scaled (ur,ui) pair-columns, zero-shifted
  per matmul slot so 16 DoubleRow matmuls accumulate one (64, 512) PSUM tile
  = 64 m's.  PSUM is copied to fp16 and DMA'd out; host stage 2 contracts
  with exact f64 conj(u) (right-side vectors carry NO quantization error).

m is sharded across the 8 NeuronCores; per-core partial losses are summed on
host (equivalent to the scalar all-reduce).
"""

import numpy as np
import ml_dtypes

M_TOTAL = 2048
N = 128
N_CORES = 8
M_LOCAL = M_TOTAL // N_CORES       # 256 m's per core
PAIRS_PER_MM = 4                   # m's per matmul (out free 512 = 4*128)
MM_PER_GROUP = 16                  # matmuls accumulated per PSUM tile
GROUP_MS = PAIRS_PER_MM * MM_PER_GROUP   # 64 m's per PSUM group
N_GROUPS = M_LOCAL // GROUP_MS     # 4
MM_PER_CHUNK = 4                   # matmul slots per input DMA (512 KiB fp8)
N_CHUNKS = M_LOCAL // (PAIRS_PER_MM * MM_PER_CHUNK)  # 16
U_SCALE = 32.0                     # keeps u entries in fp8 normal range

IN_DMA_ENGINES = ("sync", "scalar", "gpsimd")   # the only DMA-capable rings
# chunk -> ring: sync/scalar carry 6 input chunks each, gpsimd 4 plus the
# u-load and the 4 outputs (balances bytes per ring).
CHUNK_RING = [0, 1, 2, 0, 1, 2, 0, 1, 2, 0, 1, 2, 0, 1, 0, 1]
BT_BUFS = 4                        # input tile buffering depth per ring
PSUM_BUFS = 4

_CACHE: dict = {}


def _build_nc():
    """Build + compile the per-core SPMD program."""
    import concourse.bacc as bacc
    import concourse.mybir as mybir
    from concourse import tile

    f8 = mybir.dt.float8e4
    f16 = mybir.dt.float16
    f32 = mybir.dt.float32

    nc = bacc.Bacc("TRN2", target_bir_lowering=False, debug=False,
                   num_devices=N_CORES)
    # xs[c, a, q, r, j*N+b]: chunk c, slot q, plane r (0=R,1=I), pair j
    xs_in = nc.dram_tensor(
        "xs", [N_CHUNKS, N, MM_PER_CHUNK, 2, PAIRS_PER_MM * N], f8,
        kind="ExternalInput")
    # u[a, r, i*64 + col]: zero-shifted stationary plane pair for slot i
    u_in = nc.dram_tensor(
        "u", [N, 2, MM_PER_GROUP * GROUP_MS], f8, kind="ExternalInput")
    t_out = nc.dram_tensor(
        "t_out", [N_GROUPS, GROUP_MS, PAIRS_PER_MM * N], f16,
        kind="ExternalOutput")

    chunks_per_group = MM_PER_GROUP // MM_PER_CHUNK

    with tile.TileContext(nc) as tc:
        with (
            tc.tile_pool(name="bt0", bufs=BT_BUFS) as bp0,
            tc.tile_pool(name="bt1", bufs=BT_BUFS) as bp1,
            tc.tile_pool(name="bt2", bufs=BT_BUFS) as bp2,
            tc.tile_pool(name="ps", bufs=PSUM_BUFS, space="PSUM") as ppool,
            tc.tile_pool(name="st", bufs=2) as spool,
            tc.tile_pool(name="cn", bufs=1) as cpool,
        ):
            in_engines = [getattr(nc, e) for e in IN_DMA_ENGINES]
            bpools = [bp0, bp1, bp2]
            u_t = cpool.tile([N, 2, MM_PER_GROUP * GROUP_MS], f8)
            nc.gpsimd.dma_start(u_t[:], u_in[:])
            for g in range(N_GROUPS):
                psum = ppool.tile([GROUP_MS, PAIRS_PER_MM * N], f32)
                for ci in range(chunks_per_group):
                    c = g * chunks_per_group + ci
                    ring = CHUNK_RING[c]
                    bt = bpools[ring].tile(
                        [N, MM_PER_CHUNK, 2, PAIRS_PER_MM * N], f8)
                    in_engines[ring].dma_start(bt[:], xs_in[c])
                    for q in range(MM_PER_CHUNK):
                        i = ci * MM_PER_CHUNK + q
                        nc.tensor.matmul(
                            psum[:],
                            u_t[:, :, i * GROUP_MS:(i + 1) * GROUP_MS],
                            bt[:, q, :, :],
                            start=(i == 0),
                            stop=(i == MM_PER_GROUP - 1),
                            perf_mode=mybir.MatmulPerfMode.DoubleRow,
                        )
                stage = spool.tile([GROUP_MS, PAIRS_PER_MM * N], f16)
                nc.vector.tensor_copy(stage[:], psum[:])
                nc.gpsimd.dma_start(t_out[g], stage[:])
    nc.compile()
    return nc


def _get_nc():
    if "nc" not in _CACHE:
        _CACHE["nc"] = _build_nc()
    return _CACHE["nc"]


def _host_prep(theta: np.ndarray, evl: np.ndarray):
    """Eigenvector/eigenvalue prep (tiny, f64 on host)."""
    theta = np.asarray(theta, dtype=np.float64)
    evl = np.asarray(evl, dtype=np.float64)
    c0 = theta[0] + 1j * theta[1]
    evc0 = c0 / np.linalg.norm(c0)
    c1 = theta[2] + 1j * theta[3]
    c1 = c1 - np.vdot(evc0, c1) * evc0
    evc1 = c1 / np.linalg.norm(c1)
    lam = np.log1p(np.exp(evl))
    lam = lam / np.linalg.norm(lam)
    U = np.stack([evc0.real, evc0.imag, evc1.real, evc1.imag], axis=1)
    return U, lam  # f64 (128, 4), f64 (2,)


def _make_u_planes(U: np.ndarray) -> np.ndarray:
    """Zero-shifted DoubleRow stationary planes, fp8, scaled by U_SCALE.

    Slot i covers out partitions 4i..4i+3 = rows [g1_0, g2_0, g1_1, g2_1]:
      plane 0 (applied to R): [ur0, ui0, ur1, ui1]
      plane 1 (applied to I): [ui0, -ur0, ui1, -ur1]
    """
    A = U * U_SCALE
    Bp = np.stack([U[:, 1], -U[:, 0], U[:, 3], -U[:, 2]], axis=1) * U_SCALE
    u_np = np.zeros((N, 2, MM_PER_GROUP, GROUP_MS), dtype=np.float32)
    for i in range(MM_PER_GROUP):
        u_np[:, 0, i, 4 * i:4 * i + 4] = A
        u_np[:, 1, i, 4 * i:4 * i + 4] = Bp
    return np.ascontiguousarray(
        u_np.reshape(N, 2, MM_PER_GROUP * GROUP_MS)
    ).astype(ml_dtypes.float8_e4m3)


def _pack_stream(basis_re_k: np.ndarray, basis_im_k: np.ndarray) -> np.ndarray:
    """fp8-cast + pack one core's slice to the xs layout.

    m = c*16 + q*4 + j  ->  xs[c, a, q, r, j*N + b], r=0: R, r=1: I.
    """
    R = np.asarray(basis_re_k, dtype=np.float32).astype(ml_dtypes.float8_e4m3)
    I = np.asarray(basis_im_k, dtype=np.float32).astype(ml_dtypes.float8_e4m3)
    R5 = R.reshape(N_CHUNKS, MM_PER_CHUNK, PAIRS_PER_MM, N, N)
    I5 = I.reshape(N_CHUNKS, MM_PER_CHUNK, PAIRS_PER_MM, N, N)
    X = np.stack([R5, I5], axis=2)            # [c, q, r, j, a, b]
    X = np.transpose(X, (0, 4, 1, 2, 3, 5))   # [c, a, q, r, j, b]
    return np.ascontiguousarray(
        X.reshape(N_CHUNKS, N, MM_PER_CHUNK, 2, PAIRS_PER_MM * N))


def _decode(t_raw: np.ndarray, U: np.ndarray, lam: np.ndarray) -> float:
    """Host stage 2 + combine for one core's t_out. Returns partial loss."""
    # t_raw[g, 4i + x', j*128 + b], m = g*64 + i*4 + j
    G = t_raw.reshape(N_GROUPS, MM_PER_GROUP, 4, PAIRS_PER_MM, N).astype(
        np.float64)
    G = np.transpose(G, (0, 1, 3, 2, 4)).reshape(M_LOCAL, 4, N) / U_SCALE
    u0 = U[:, 0] + 1j * U[:, 1]
    u1 = U[:, 2] + 1j * U[:, 3]
    F0 = (G[:, 0, :] + 1j * G[:, 1, :]) @ np.conj(u0)
    F1 = (G[:, 2, :] + 1j * G[:, 3, :]) @ np.conj(u1)
    v = lam[0] * F0 - lam[1] * F1
    return float(np.sum(v.real ** 2 + v.imag ** 2))


def _make_in_maps(basis_re, basis_im, theta, evl):
    U, lam = _host_prep(theta, evl)
    u_packed = _make_u_planes(U)
    in_maps = []
    for k in range(N_CORES):
        sl = slice(k * M_LOCAL, (k + 1) * M_LOCAL)
        in_maps.append({
            "xs": _pack_stream(basis_re[sl], basis_im[sl]),
            "u": u_packed,
        })
    return in_maps, U, lam


def _run_device(in_maps, **kwargs):
    from concourse.bass_utils import run_bass_kernel_spmd
    nc = _get_nc()
    return run_bass_kernel_spmd(nc, in_maps, list(range(N_CORES)), **kwargs)


def kernel(basis_re, basis_im, theta, evl) -> np.ndarray:
    in_maps, U, lam = _make_in_maps(basis_re, basis_im, theta, evl)
    res = _run_device(in_maps)
    total = 0.0
    for k in range(N_CORES):
        total += _decode(res.results[k]["t_out"], U, lam)
    return np.float32(total)
